# revision 17
# baseline (speedup 1.0000x reference)
"""Trainium2 Bass kernel for nn_AttnSeq2Seq (2-layer LSTM encoder + attention decoder).

Sharding: pure data parallelism — batch 1024 split as 8 x 128 (one shard per core),
weights replicated. B_loc=128 rides the partition dimension everywhere.

Per-core pipeline:
  Encoder (336 steps, fused 2 layers): batch-stationary matmuls
    (lhsT = transposed hidden state [K,128], moving = fp16 weight rows, N=512),
    PSUM fp32 accumulation, ACT sigmoid/tanh gates, DVE cell update,
    PE transposes for the next step's stationary. h1 outputs are flushed to DRAM
    as encT [3][128 d][128 b][336 l] fp16 (d on partitions).
  Decoder (18 steps): scores via per-b M=1 matmuls (g_b stationary, enc moving)
    accumulated at 32-aligned PSUM partitions; ACT exp extraction; DMA scatter;
    ctx via DVE scalar_tensor_tensor with accum_out against a DMA-broadcast
    alpha row; small dense matmuls for dec_in/LSTM/output head.
All matmul operands fp16 (values are tanh/sigmoid-bounded), fp32 accumulation.
"""
import os
import numpy as np
from contextlib import ExitStack

import concourse.bass as bass
import concourse.tile as tile
from concourse import bacc, mybir, bass_utils, masks
from concourse.tile import add_dep_helper

f32 = mybir.dt.float32
f16 = mybir.dt.float16
AF = mybir.ActivationFunctionType
OP = mybir.AluOpType

B, DX, H = 1024, 8, 384
L = int(os.environ.get("K_L", "336"))
HZ = int(os.environ.get("K_HZ", "18"))
NC = 8
BL = B // NC          # 128 per core
G4 = 4 * H            # 1536
SPI = 16              # encoder steps per loop iteration
N_ITER = L // SPI     # 21
EXP_SHIFT = -4.0      # exp(s + EXP_SHIFT): cancels in softmax, keeps fp16 in range

_cache = {}


def _build():
    nc = bacc.Bacc("TRN2", target_bir_lowering=False, debug=False)

    # ---------------- DRAM I/O (all host-prepped layouts) ----------------
    d_xT = nc.dram_tensor("xT", [L, DX + 1, BL], f16, kind="ExternalInput").ap()
    d_wih0 = nc.dram_tensor("wih0", [DX + 1, G4], f16, kind="ExternalInput").ap()
    d_whh0 = nc.dram_tensor("whh0", [3, 128, G4], f16, kind="ExternalInput").ap()
    d_wih1 = nc.dram_tensor("wih1", [3, 128, G4], f16, kind="ExternalInput").ap()
    d_whh1 = nc.dram_tensor("whh1", [3, 128, G4], f16, kind="ExternalInput").ap()
    d_bias1 = nc.dram_tensor("bias1", [1, G4], f16, kind="ExternalInput").ap()
    d_wa = nc.dram_tensor("wa", [3, 3, 128, 128], f16, kind="ExternalInput").ap()
    d_dinw = nc.dram_tensor("dinw", [3, 128, H], f16, kind="ExternalInput").ap()
    d_dinwt = nc.dram_tensor("dinwt", [6, H], f16, kind="ExternalInput").ap()
    d_dwih = nc.dram_tensor("dwih", [3, 128, G4], f16, kind="ExternalInput").ap()
    d_dwhh = nc.dram_tensor("dwhh", [3, 128, G4], f16, kind="ExternalInput").ap()
    d_dbias = nc.dram_tensor("dbias", [1, G4], f16, kind="ExternalInput").ap()
    d_outw = nc.dram_tensor("outw", [3, 128, 1], f16, kind="ExternalInput").ap()
    d_outb = nc.dram_tensor("outb", [1, 1], f32, kind="ExternalInput").ap()
    d_featT = nc.dram_tensor("featT", [HZ, 4, BL], f16, kind="ExternalInput").ap()
    d_y = nc.dram_tensor("y", [BL, HZ], f32, kind="ExternalOutput").ap()
    d_h1T = nc.dram_tensor("h1Tdump", [3, 128, 128], f16, kind="ExternalOutput").ap()
    DBG = os.environ.get("DBG_ONESTEP") == "1"
    DBG2 = os.environ.get("DBG_DUMP") == "1"
    if DBG2:
        d_aU2 = nc.dram_tensor("aU2", [HZ, 128, L], f16, kind="ExternalOutput").ap()
        d_gT2 = nc.dram_tensor("gT2", [HZ, 128, H], f16, kind="ExternalOutput").ap()
        d_ctx2 = nc.dram_tensor("ctx2", [HZ, 3, 128, 128], f32, kind="ExternalOutput").ap()
        d_din2 = nc.dram_tensor("din2", [HZ, 128, H], f32, kind="ExternalOutput").ap()
        d_hd2 = nc.dram_tensor("hd2", [HZ, 128, H], f32, kind="ExternalOutput").ap()
    if DBG:
        d_alphaU = nc.dram_tensor("alphaUdump", [128, L], f32, kind="ExternalOutput").ap()
        d_gT = nc.dram_tensor("gTdump", [128, H], f32, kind="ExternalOutput").ap()
        d_ctxT = nc.dram_tensor("ctxTdump", [3, 128, 128], f32, kind="ExternalOutput").ap()
    d_encT = nc.dram_tensor("encT", [3, 128, BL, L], f16, kind="ExternalOutput").ap()

    with tile.TileContext(nc) as tc, ExitStack() as ctx:
        wp = ctx.enter_context(tc.tile_pool(name="weights", bufs=1))
        st = ctx.enter_context(tc.tile_pool(name="state", bufs=1))
        gp = ctx.enter_context(tc.tile_pool(name="gates", bufs=2))
        xp = ctx.enter_context(tc.tile_pool(name="xin", bufs=2))
        fb = ctx.enter_context(tc.tile_pool(name="flush", bufs=2))
        ps = ctx.enter_context(tc.tile_pool(name="psum", bufs=2, space="PSUM"))
        dp = ctx.enter_context(tc.tile_pool(name="dram", bufs=2 * HZ, space="DRAM"))
        sp = ctx.enter_context(tc.tile_pool(name="stream", bufs=2))
        ap_ = ctx.enter_context(tc.tile_pool(name="alphas", bufs=4))

        # ---------------- load weights ----------------
        wih0 = wp.tile([DX + 1, G4], f16, name="wih0"); nc.sync.dma_start(wih0[:], d_wih0)
        whh0 = [wp.tile([128, G4], f16, tag=f"whh0{k}", name=f"whh0{k}") for k in range(3)]
        whh1 = [wp.tile([128, G4], f16, tag=f"whh1{k}", name=f"whh1{k}") for k in range(3)]
        wih1 = [wp.tile([128, G4], f16, tag=f"wih1{k}", name=f"wih1{k}") for k in range(3)]
        for k in range(3):
            nc.sync.dma_start(whh0[k][:], d_whh0[k])
            nc.sync.dma_start(whh1[k][:], d_whh1[k])
            nc.sync.dma_start(wih1[k][:], d_wih1[k])
        bias1 = wp.tile([1, G4], f16, name="bias1"); nc.sync.dma_start(bias1[:], d_bias1)
        wa = [[wp.tile([128, 128], f16, tag=f"wa{k}{m}", name=f"wa{k}{m}") for m in range(3)] for k in range(3)]
        for k in range(3):
            for m in range(3):
                nc.sync.dma_start(wa[k][m][:], d_wa[k, m])
        dinw = [wp.tile([128, H], f16, tag=f"dinw{k}", name=f"dinw{k}") for k in range(3)]
        for k in range(3):
            nc.sync.dma_start(dinw[k][:], d_dinw[k])
        dinwt = wp.tile([6, H], f16, name="dinwt"); nc.sync.dma_start(dinwt[:], d_dinwt)
        dwih = [wp.tile([128, G4], f16, tag=f"dwih{k}", name=f"dwih{k}") for k in range(3)]
        dwhh = [wp.tile([128, G4], f16, tag=f"dwhh{k}", name=f"dwhh{k}") for k in range(3)]
        for k in range(3):
            nc.sync.dma_start(dwih[k][:], d_dwih[k])
            nc.sync.dma_start(dwhh[k][:], d_dwhh[k])
        dbias = wp.tile([1, G4], f16, name="dbias"); nc.sync.dma_start(dbias[:], d_dbias)
        outw = [wp.tile([128, 1], f16, tag=f"outw{k}", name=f"outw{k}") for k in range(3)]
        for k in range(3):
            nc.sync.dma_start(outw[k][:], d_outw[k])
        outb = wp.tile([1, 1], f32, name="outb"); nc.sync.dma_start(outb[:], d_outb)

        ident = wp.tile([128, 128], f32)
        masks.make_identity(nc, ident[:])
        ones1 = wp.tile([1, 128], f16, name="ones1"); nc.gpsimd.memset(ones1[:], 1.0)
        expb = wp.tile([1, 1], f32, name="expb"); nc.gpsimd.memset(expb[:], EXP_SHIFT)
        obrep = wp.tile([128, 1], f32, name="obrep")
        nc.sync.dma_start(obrep[:], d_outb[0:1, :].partition_broadcast(128))

        # ---------------- persistent state ----------------
        h0T = [[st.tile([128, 128], f16, tag=f"h0T{p}{k}", name=f"h0T{p}{k}") for k in range(3)] for p in range(2)]
        h1T = [[st.tile([128, 128], f16, tag=f"h1T{p}{k}", name=f"h1T{p}{k}") for k in range(3)] for p in range(2)]
        c0 = st.tile([128, H], f32, name="c0"); c1 = st.tile([128, H], f32, name="c1")
        for p in range(2):
            for k in range(3):
                nc.gpsimd.memset(h0T[p][k][:], 0.0)
                nc.gpsimd.memset(h1T[p][k][:], 0.0)
        nc.gpsimd.memset(c0[:], 0.0)
        nc.gpsimd.memset(c1[:], 0.0)

        def lstm_gates(zp, c, hname):
            """gates from z PSUM [128,1536]; returns h fp32 [128,H]."""
            i_s = gp.tile([128, H], f16, tag="i_s", name="i_s")
            f_s = gp.tile([128, H], f16, tag="f_s")
            g_t = gp.tile([128, H], f16, tag="g_t")
            o_s = gp.tile([128, H], f16, tag="o_s")
            nc.scalar.activation(i_s[:], zp[:, 0:H], AF.Sigmoid)
            nc.scalar.activation(f_s[:], zp[:, H:2 * H], AF.Sigmoid)
            nc.scalar.activation(g_t[:], zp[:, 2 * H:3 * H], AF.Tanh)
            nc.scalar.activation(o_s[:], zp[:, 3 * H:4 * H], AF.Sigmoid)
            t1 = gp.tile([128, H], f16, tag="t1")
            nc.vector.tensor_tensor(t1[:], i_s[:], g_t[:], OP.mult)
            t2 = gp.tile([128, H], f32, tag="t2")
            nc.vector.tensor_tensor(t2[:], f_s[:], c[:], OP.mult)
            nc.vector.tensor_tensor(c[:], t1[:], t2[:], OP.add)
            tc_t = gp.tile([128, H], f16, tag="tc_t")
            nc.scalar.activation(tc_t[:], c[:], AF.Tanh)
            h = gp.tile([128, H], f32, tag=hname, name=hname)
            nc.vector.tensor_tensor(h[:], o_s[:], tc_t[:], OP.mult)
            return h

        def transpose_to(h, dstT, flushbuf=None, s=None):
            """h fp32 [128,H] -> 3 PE transposes -> fp16 dstT[k] [128,128].
            Optionally also write fp16 into flushbuf[:, k, :, s]."""
            for k in range(3):
                pt = ps.tile([128, 128], f32, tag="tr")
                nc.tensor.transpose(pt[:], h[:, 128 * k:128 * (k + 1)], ident[:])
                nc.scalar.copy(dstT[k][:], pt[:])
                if flushbuf is not None:
                    nc.vector.tensor_copy(flushbuf[:, k, :, s], pt[:])

        # ---------------- encoder loop ----------------
        def enc_body(i):
            xbuf = xp.tile([DX + 1, SPI * BL], f16)
            xv = xbuf[:].rearrange("p (s b) -> p s b", s=SPI)
            nc.sync.dma_start(xv, d_xT[bass.ds(i * SPI, SPI)].transpose([1, 0, 2]))
            flush = fb.tile([128, 3 * BL * SPI], f16)
            fv = flush[:].rearrange("p (c b s) -> p c b s", c=3, b=BL)
            for s in range(SPI):
                par, nxt = s % 2, (s + 1) % 2
                # --- layer 0
                z0 = ps.tile([128, G4], f32, tag="z")
                for n in range(3):
                    sl = slice(512 * n, 512 * (n + 1))
                    nc.tensor.matmul(z0[:, sl], xbuf[:, bass.ts(s, BL)], wih0[:, sl],
                                     start=True, stop=False)
                    for k in range(3):
                        nc.tensor.matmul(z0[:, sl], h0T[par][k][:], whh0[k][:, sl],
                                         start=False, stop=(k == 2))
                h0 = lstm_gates(z0, c0, "h0")
                transpose_to(h0, h0T[nxt])
                # --- layer 1
                z1 = ps.tile([128, G4], f32, tag="z")
                for n in range(3):
                    sl = slice(512 * n, 512 * (n + 1))
                    for k in range(3):
                        nc.tensor.matmul(z1[:, sl], h0T[nxt][k][:], wih1[k][:, sl],
                                         start=(k == 0), stop=False)
                    for k in range(3):
                        nc.tensor.matmul(z1[:, sl], h1T[par][k][:], whh1[k][:, sl],
                                         start=False, stop=False)
                    nc.tensor.matmul(z1[:, sl], ones1[:], bias1[:, sl],
                                     start=False, stop=True)
                h1 = lstm_gates(z1, c1, "h1")
                transpose_to(h1, h1T[nxt], fv, s)
            for cch in range(3):
                fl = nc.sync.dma_start(
                    d_encT[cch][:, :, bass.ds(i * SPI, SPI)], fv[:, cch, :, :])
                flush_insts.append(fl.ins)

        flush_insts = []
        if N_ITER > 1:
            with tc.For_i(0, N_ITER, 1) as iv:
                enc_body(iv)
        else:
            enc_body(0)

        # decoder initial state: h1T parity 0 (336 % 2 == 0), c/y zeros
        hdT = h1T[0]
        for k in range(3):
            nc.sync.dma_start(d_h1T[k], hdT[k][:])
        cd = st.tile([128, H], f32, tag="cd", name="cd"); nc.gpsimd.memset(cd[:], 0.0)
        yT = st.tile([1, 128], f16, name="yT"); nc.gpsimd.memset(yT[:], 0.0)
        tail = st.tile([6, 128], f16, name="tail"); nc.gpsimd.memset(tail[:], 1.0)  # row5 stays 1

        # ---------------- decoder loop ----------------
        def dec_body(t):
            # feat rows into tail rows 1..4
            nc.sync.dma_start(tail[1:5, :], d_featT[bass.ds(t, 1)].squeeze(0))
            nc.scalar.copy(tail[0:1, :], yT[:])
            # g = Wa.T contraction: gT [128 d', (m,b)]
            gps = ps.tile([128, H], f32, tag="z")
            for m in range(3):
                for k in range(3):
                    nc.tensor.matmul(gps[:, bass.ts(m, 128)], wa[k][m][:], hdT[k][:],
                                     start=(k == 0), stop=(k == 2))
            gT = gp.tile([128, H], f16, tag="gT")
            nc.scalar.copy(gT[:], gps[:])

            alphaU = ap_.tile([128, L], f16, tag="alphaU")
            alphaN_d = dp.tile([BL, L], f16, tag="alphaN_d")
            ctxT = [gp.tile([128, 128], f32, tag=f"ctxT{k}", name=f"ctxT{k}") for k in range(3)]
            # 64 rounds of 2 consecutive b; PSUM slots bank-aligned (512 f32)
            for r in range(64):
                j = r // 16
                g8 = r // 4  # stream tile index (8 b's each)
                if r % 4 == 0:
                    stile = sp.tile([128, 3 * 8 * L], f16, tag="enc_stream")
                    sv = stile[:].rearrange("p (c b l) -> p c b l", c=3, b=8)
                    sdma = nc.sync.dma_start(
                        sv, d_encT[:, :, bass.ds(8 * g8, 8), :].transpose([1, 0, 2, 3]))
                    for fli in flush_insts:
                        add_dep_helper(sdma.ins, fli, sync=True,
                                       reason="enc stream read waits on flush")
                    cur_sv = sv
                spsum = ps.tile([128, 1024], f32, tag="z")
                for idx in range(2):
                    b = 2 * r + idx
                    bloc = b - 8 * g8
                    for k in range(3):
                        nc.tensor.matmul(
                            spsum[32 * j:32 * j + 1, 512 * idx:512 * idx + L],
                            gT[:, 128 * k + b:128 * k + b + 1],
                            cur_sv[:, k, bloc, :],
                            start=(k == 0), stop=(k == 2), tile_position=(0, 32 * j))
                erow = ap_.tile([1, 2 * L], f16, tag="erow")
                spv = spsum[32 * j:32 * j + 1, :].rearrange("p (i l) -> p i l", i=2)
                nc.scalar.activation(erow[:], spv[:, :, 0:L], AF.Exp,
                                     bias=expb[0:1, :], scale=1.0)
                nc.sync.dma_start(alphaU[2 * r:2 * r + 2, :], erow[:])
                wr_inst = nc.sync.dma_start(alphaN_d[2 * r:2 * r + 2, :], erow[:])
                # ctx accumulation for these 2 b (unnormalized alpha)
                for idx in range(2):
                    b = 2 * r + idx
                    bloc = b - 8 * g8
                    arep = ap_.tile([128, L], f16, tag="arep")
                    rd_inst = nc.sync.dma_start(
                        arep[:], alphaN_d[b:b + 1, :].partition_broadcast(128))
                    add_dep_helper(rd_inst.ins, wr_inst.ins, sync=True,
                                   reason="bcast read waits on alphaN dram write")
                    for k in range(3):
                        sc = ap_.tile([128, L], f16, tag="sttscr")
                        nc.vector.scalar_tensor_tensor(
                            out=sc[:], in0=cur_sv[:, k, bloc, :], scalar=1.0,
                            in1=arep[:], op0=OP.mult, op1=OP.mult,
                            accum_out=ctxT[k][:, b:b + 1])
            # softmax denominator and ctx normalization (1/sum broadcast via DRAM)
            if DBG:
                au32 = gp.tile([128, L], f32, tag="au32", name="au32")
                nc.vector.tensor_copy(au32[:], alphaU[:])
                nc.sync.dma_start(d_alphaU, au32[:])
                gT32 = gp.tile([128, H], f32, tag="gT32", name="gT32")
                nc.vector.tensor_copy(gT32[:], gT[:])
                nc.sync.dma_start(d_gT, gT32[:])
            se = gp.tile([128, 1], f32, tag="se")
            nc.vector.tensor_reduce(se[:], alphaU[:], mybir.AxisListType.X, op=OP.add)
            rcp = gp.tile([128, 1], f32, tag="rcp")
            nc.vector.reciprocal(rcp[:], se[:])
            rcp_d = dp.tile([BL, 1], f32, tag="rcp_d")
            wr2 = nc.sync.dma_start(rcp_d[:, :], rcp[:])
            rrep = gp.tile([128, 128], f32, tag="rrep")
            rd2 = nc.sync.dma_start(rrep[:],
                              rcp_d[:, :].rearrange("b one -> one b").partition_broadcast(128))
            add_dep_helper(rd2.ins, wr2.ins, sync=True, reason="rrep bcast waits on rcp write")
            dinT = [gp.tile([128, 128], f16, tag=f"dinT{k}", name=f"dinT{k}") for k in range(3)]
            for k in range(3):
                nc.vector.tensor_tensor(ctxT[k][:], ctxT[k][:], rrep[:], OP.mult)
                nc.vector.tensor_copy(dinT[k][:], ctxT[k][:])
            if DBG:
                for k in range(3):
                    nc.sync.dma_start(d_ctxT[k], ctxT[k][:])
            if DBG2:
                nc.sync.dma_start(d_aU2[t], alphaU[:])
                nc.sync.dma_start(d_gT2[t], gT[:])
                for k in range(3):
                    nc.sync.dma_start(d_ctx2[t, k], ctxT[k][:])
            # dec_in: din = relu(dinW' @ [ctx;y;feat;ones])
            dps = ps.tile([128, H], f32, tag="z")
            for k in range(3):
                nc.tensor.matmul(dps[:], dinT[k][:], dinw[k][:],
                                 start=(k == 0), stop=False)
            nc.tensor.matmul(dps[:], tail[:], dinwt[:], start=False, stop=True)
            din_a = gp.tile([128, H], f32, tag="din_a")
            nc.scalar.activation(din_a[:], dps[:], AF.Relu)
            daT = [gp.tile([128, 128], f16, tag=f"daT{k}", name=f"daT{k}") for k in range(3)]
            for k in range(3):
                pt = ps.tile([128, 128], f32, tag="tr")
                nc.tensor.transpose(pt[:], din_a[:, bass.ts(k, 128)], ident[:])
                nc.scalar.copy(daT[k][:], pt[:])
            # decoder LSTM
            zp = ps.tile([128, G4], f32, tag="z")
            for n in range(3):
                sl = slice(512 * n, 512 * (n + 1))
                for k in range(3):
                    nc.tensor.matmul(zp[:, sl], daT[k][:], dwih[k][:, sl],
                                     start=(k == 0), stop=False)
                for k in range(3):
                    nc.tensor.matmul(zp[:, sl], hdT[k][:], dwhh[k][:, sl],
                                     start=False, stop=False)
                nc.tensor.matmul(zp[:, sl], ones1[:], dbias[:, sl],
                                 start=False, stop=True)
            hd = lstm_gates(zp, cd, "hd")
            if DBG2:
                nc.sync.dma_start(d_din2[t], din_a[:])
                nc.sync.dma_start(d_hd2[t], hd[:])
            transpose_to(hd, hdT)
            # y[b] = out_W . h_b + b : lhsT=hdT (M=128 batch), rhs=outw (N=1)
            yps = ps.tile([128, 1], f32, tag="tr")
            for k in range(3):
                nc.tensor.matmul(yps[:], hdT[k][:], outw[k][:],
                                 start=(k == 0), stop=(k == 2))
            ybs = gp.tile([128, 1], f32, tag="ybs")
            nc.scalar.activation(ybs[:], yps[:], AF.Identity, bias=obrep[:, 0:1])
            nc.sync.dma_start(d_y[:, bass.ds(t, 1)], ybs[:])
            ytp = ps.tile([128, 128], f32, tag="tr")
            nc.tensor.transpose(ytp[0:1, :], ybs[:], ident[:])
            nc.scalar.copy(yT[:], ytp[0:1, :])

        if DBG:
            dec_body(0)
        elif os.environ.get("DEC_FORI") == "1":
            with tc.For_i(0, HZ, 1) as tv:
                dec_body(tv)
        else:
            for tv in range(HZ):
                dec_body(tv)

    nc.compile()
    return nc


def _prep(inputs):
    """Host-side packing of all weights/inputs into device layouts."""
    g = {k: np.asarray(v, np.float32) for k, v in inputs.items()}
    h16 = lambda a: np.ascontiguousarray(a, dtype=np.float16)
    pr = {}
    pr["wih0"] = h16(np.concatenate([g["enc_Wih0"].T,
                                     (g["enc_bih0"] + g["enc_bhh0"])[None, :]], 0))
    pr["whh0"] = h16(g["enc_Whh0"].T.reshape(3, 128, G4))
    pr["wih1"] = h16(g["enc_Wih1"].T.reshape(3, 128, G4))
    pr["whh1"] = h16(g["enc_Whh1"].T.reshape(3, 128, G4))
    pr["bias1"] = h16((g["enc_bih1"] + g["enc_bhh1"])[None, :])
    wa = g["Wa"]  # [384, 384]
    pr["wa"] = h16(wa.reshape(3, 128, 3, 128).transpose(0, 2, 1, 3))
    W = g["dec_in_W"]; bvec = g["dec_in_b"]
    Wp = np.concatenate([W[:, 5:389], W[:, 0:1], W[:, 1:5], bvec[:, None]], 1)  # [384, 390]
    WpT = Wp.T  # [390, 384]
    pr["dinw"] = h16(WpT[:384].reshape(3, 128, H))
    pr["dinwt"] = h16(WpT[384:390])
    pr["dwih"] = h16(g["dec_Wih"].T.reshape(3, 128, G4))
    pr["dwhh"] = h16(g["dec_Whh"].T.reshape(3, 128, G4))
    pr["dbias"] = h16((g["dec_bih"] + g["dec_bhh"])[None, :])
    pr["outw"] = h16(g["out_W"].T.reshape(3, 128, 1))
    pr["outb"] = np.ascontiguousarray(g["out_b"].reshape(1, 1), np.float32)
    return g, pr


def kernel(**inputs):
    if "nc" not in _cache:
        _cache["nc"] = _build()
    nc = _cache["nc"]
    g, pr = _prep(inputs)
    in_maps = []
    for c in range(NC):
        sl = slice(c * BL, (c + 1) * BL)
        x = g["x"][sl]                     # [128, 336, 8]
        xe = np.concatenate([x, np.ones((BL, L, 1), np.float32)], 2)  # [128,336,9]
        m = dict(pr)
        m["xT"] = np.ascontiguousarray(xe.transpose(1, 2, 0), np.float16)
        m["featT"] = np.ascontiguousarray(
            g["future_feats"][sl].transpose(1, 2, 0), np.float16)
        in_maps.append(m)
    res = bass_utils.run_bass_kernel_spmd(nc, in_maps, core_ids=list(range(NC)))
    out = np.concatenate([res.results[c]["y"] for c in range(NC)], 0)  # [1024, 18]
    return np.ascontiguousarray(out[:, :, None], np.float32)
